# revision 11
# baseline (speedup 1.0000x reference)
"""Trainium2 Bass kernel for the additive-attention glimpse module.

Math (per batch b):
    qp  = query @ Wq.T + bq                       # [E]
    cp  = context @ Wc.T + bc                     # [N, E]
    comb = tanh(qp + cp)                          # [N, E]
    attn = comb @ Wo.T (+ bo, softmax-invariant)  # [N, G]
    w    = softmax(attn, axis=N)                  # [N, G]
    out  = (w.T @ context).reshape(G*Cd)          # [G*Cd]

Shapes: B=256, N=196, Cd=2048, Qd=E=1024, G=8.

Strategy: data-parallel over B across 8 cores (32 batches each). On each
core everything is kept feature-on-partition ("transposed" layout) so the
dominant matmul (context @ Wc.T, ~26 GFLOP/core) runs as
cp.T[e, r] = WcT.T @ ctx.T with bf16 operands at 1 cycle/row. ctx.T tiles
come from HBM via the DMA xbar transpose (bf16-only path), so the
TensorEngine spends no cycles transposing the big tensor. The natural
[n, c] layout (needed by the glimpse matmul, which contracts over n) is a
second, plain DMA of the same bf16 array.
"""

import numpy as np
import ml_dtypes

BF16 = ml_dtypes.bfloat16

B_FULL = 256
N_CTX = 196
CD = 2048
QD = 1024
E = 1024
G = 8
N_CORES = 8
B_LOC = B_FULL // N_CORES  # 32

SLAB_B = 4          # batches per transpose-DMA slab (rows 4*196=784, /16 ok)
CHUNK_B = 2         # batches per compute chunk (rows 392 <= 512 psum bank)
CHUNK_R = CHUNK_B * N_CTX  # 392


def build_nc(b_loc=B_LOC, reps=1, rep_scales=None):
    """Build the single-core Bass/Tile graph (SPMD: same graph on all cores).

    reps>1 repeats the whole computation (same inputs -> same outputs)
    inside one NEFF; used only for wall-clock HW timing, since per-execute
    RPC overhead in this container is ~100ms.
    """
    import concourse.mybir as mybir
    import concourse.tile as tile
    from concourse import bacc
    from concourse.masks import make_identity

    f32 = mybir.dt.float32
    bf16 = mybir.dt.bfloat16
    Act = mybir.ActivationFunctionType
    Alu = mybir.AluOpType

    assert b_loc % SLAB_B == 0
    n_slab = b_loc // SLAB_B
    R = b_loc * N_CTX

    nc = bacc.Bacc("TRN2", target_bir_lowering=False, debug=False,
                   num_devices=N_CORES)

    ctx = nc.dram_tensor("ctx", [R, CD], bf16, kind="ExternalInput").ap()
    qT = nc.dram_tensor("qT", [QD, b_loc], f32, kind="ExternalInput").ap()
    WqT = nc.dram_tensor("WqT", [QD, E], f32, kind="ExternalInput").ap()
    WcT = nc.dram_tensor("WcT", [CD, E], bf16, kind="ExternalInput").ap()
    WoT = nc.dram_tensor("WoT", [E, G], bf16, kind="ExternalInput").ap()
    bqc = nc.dram_tensor("bqc", [128, E // 128], f32, kind="ExternalInput").ap()
    out = nc.dram_tensor("out", [b_loc, G * CD], f32, kind="ExternalOutput").ap()

    NE = E // 128    # 8 e-tiles
    NCC = CD // 128  # 16 c-tiles
    NQ = QD // 128   # 8 q-tiles

    with tile.TileContext(nc) as tc:
        with (
            tc.tile_pool(name="const", bufs=1) as const_pool,
            tc.tile_pool(name="xt", bufs=2) as xt_pool,
            tc.tile_pool(name="nat", bufs=2) as nat_pool,
            tc.tile_pool(name="comb", bufs=2) as comb_pool,
            tc.tile_pool(name="sm", bufs=8) as sm_pool,
            tc.tile_pool(name="wl", bufs=8) as wl_pool,
            tc.tile_pool(name="outb", bufs=2) as outb_pool,
            tc.tile_pool(name="pcp", bufs=2, space="PSUM") as pc_pool,
            tc.tile_pool(name="pat", bufs=2, space="PSUM") as pa_pool,
            tc.tile_pool(name="ptr", bufs=2, space="PSUM") as pt_pool,
            tc.tile_pool(name="pgl", bufs=2, space="PSUM") as pg_pool,
        ):
            # ---- persistent constants ----
            wc_sb = const_pool.tile([128, NCC, E], bf16)
            nc.sync.dma_start(wc_sb[:], WcT.rearrange("(k p) e -> p k e", p=128))
            wo_sb = const_pool.tile([128, NE, G], bf16)
            nc.sync.dma_start(wo_sb[:], WoT.rearrange("(k p) g -> p k g", p=128))
            bqc_sb = const_pool.tile([128, NE], f32)
            nc.sync.dma_start(bqc_sb[:], bqc[:])
            ident = const_pool.tile([128, 128], bf16)
            make_identity(nc, ident[:])

            qpb_sb = const_pool.tile([128, NE, b_loc], f32, tag="qpb")

            def one_pass(out_scale=1.0):
                # ---- qp = Wq @ query.T (+bq+bc), kept as [e, b] ----
                qt_sb = wq_pool.tile([128, NQ, b_loc], f32, tag="qt")
                nc.sync.dma_start(qt_sb[:], qT.rearrange("(k p) b -> p k b", p=128))
                for e in range(NE):
                    wq_sb = wq_pool.tile([128, NQ, 128], f32, tag="wqe")
                    nc.sync.dma_start(
                        wq_sb[:],
                        WqT[:, e * 128:(e + 1) * 128].rearrange(
                            "(k p) m -> p k m", p=128))
                    pq = pc_pool.tile([128, b_loc], f32, tag="pcp")
                    for k in range(NQ):
                        nc.tensor.matmul(
                            pq[:], wq_sb[:, k, :],
                            qt_sb[:, k, :], start=(k == 0), stop=(k == NQ - 1),
                        )
                    nc.vector.tensor_scalar_add(
                        qpb_sb[:, e, :], pq[:], bqc_sb[:, e:e + 1])

                # ---- main loop over 4-batch slabs ----
                for s in range(n_slab):
                    r0 = s * SLAB_B * N_CTX
                    xt = xt_pool.tile([128, NCC, SLAB_B * N_CTX], bf16, tag="xt")
                    for c in range(NCC):
                        nc.sync.dma_start_transpose(
                            xt[:, c, :], ctx[r0:r0 + SLAB_B * N_CTX,
                                             c * 128:(c + 1) * 128])

                    nat_a = nat_pool.tile([128, SLAB_B, CD], bf16, tag="nat_a")
                    nat_b = nat_pool.tile([68, SLAB_B, CD], bf16, tag="nat_b")
                    for j in range(SLAB_B):
                        rb = r0 + j * N_CTX
                        nc.sync.dma_start(nat_a[:, j, :], ctx[rb:rb + 128, :])
                        nc.sync.dma_start(nat_b[:, j, :],
                                          ctx[rb + 128:rb + N_CTX, :])

                    pa_tiles = []
                    for h in range(SLAB_B // CHUNK_B):
                        b0 = s * SLAB_B + h * CHUNK_B
                        rc = h * CHUNK_R  # offset inside the slab

                        # cp.T tiles + fused bias/tanh -> comb.T (bf16)
                        comb = comb_pool.tile([128, NE, CHUNK_R], bf16, tag="comb")
                        for e in range(NE):
                            pc = pc_pool.tile([128, CHUNK_R], f32, tag="pcp")
                            for c in range(NCC):
                                nc.tensor.matmul(
                                    pc[:], wc_sb[:, c, e * 128:(e + 1) * 128],
                                    xt[:, c, rc:rc + CHUNK_R],
                                    start=(c == 0), stop=(c == NCC - 1),
                                )
                            for j in range(CHUNK_B):
                                nc.scalar.activation(
                                    comb[:, e, j * N_CTX:(j + 1) * N_CTX],
                                    pc[:, j * N_CTX:(j + 1) * N_CTX],
                                    Act.Tanh,
                                    bias=qpb_sb[:, e, b0 + j:b0 + j + 1],
                                )

                        # attn.T = WoT.T @ comb.T  -> [G, 392]
                        pa = pa_pool.tile([G, CHUNK_R], f32, tag="pat")
                        for e in range(NE):
                            nc.tensor.matmul(
                                pa[:], wo_sb[:, e, :], comb[:, e, :],
                                start=(e == 0), stop=(e == NE - 1),
                            )
                        pa_tiles.append(pa)

                    # per-batch softmax + weight transpose for the whole slab
                    wls, rss = [], []
                    for j in range(SLAB_B):
                        pa = pa_tiles[j // CHUNK_B]
                        jj = j % CHUNK_B
                        seg = pa[:, jj * N_CTX:(jj + 1) * N_CTX]
                        nmx = sm_pool.tile([G, 1], f32, tag="nmx")
                        nc.vector.tensor_reduce(
                            nmx[:], seg, axis=mybir.AxisListType.X,
                            op=Alu.max, negate=True)
                        wex = sm_pool.tile([G, N_CTX], bf16, tag="wex")
                        ssum = sm_pool.tile([G, 1], f32, tag="ssum")
                        nc.scalar.activation(
                            wex[:], seg, Act.Exp, bias=nmx[:],
                            accum_out=ssum[:])
                        rs = sm_pool.tile([G, 1], f32, tag="rs")
                        nc.vector.reciprocal(rs[:], ssum[:])
                        if out_scale != 1.0:
                            nc.vector.tensor_scalar_mul(
                                rs[:], rs[:], float(out_scale))
                        rss.append(rs)

                        wla = wl_pool.tile([128, G], bf16, tag="wla")
                        wlb = wl_pool.tile([68, G], bf16, tag="wlb")
                        pta = pt_pool.tile([128, G], bf16, tag="ptr")
                        nc.tensor.transpose(pta[:], wex[:, 0:128],
                                            ident[:G, :G])
                        nc.vector.tensor_copy(wla[:], pta[:])
                        ptb = pt_pool.tile([68, G], bf16, tag="ptr")
                        nc.tensor.transpose(ptb[:], wex[:, 128:N_CTX],
                                            ident[:G, :G])
                        nc.vector.tensor_copy(wlb[:], ptb[:])
                        wls.append((wla, wlb))

                    # glimpse for 4 batches concurrently via PE column tiling:
                    # batch j occupies column group j (out partitions 32j..32j+7)
                    outb = outb_pool.tile([128, CD], f32, tag="outb")
                    for cc in range(CD // 512):
                        pg = pg_pool.tile([128, 512], f32, tag="pgl")
                        for j in range(SLAB_B):
                            nc.tensor.matmul(
                                pg[32 * j:32 * j + G, :], wls[j][0][:],
                                nat_a[:, j, cc * 512:(cc + 1) * 512],
                                start=True, stop=False,
                                tile_position=(0, 32 * j),
                                skip_group_check=True)
                        for j in range(SLAB_B):
                            nc.tensor.matmul(
                                pg[32 * j:32 * j + G, :], wls[j][1][:],
                                nat_b[:, j, cc * 512:(cc + 1) * 512],
                                start=False, stop=True,
                                tile_position=(0, 32 * j),
                                skip_group_check=True)
                        for j in range(SLAB_B):
                            nc.vector.tensor_scalar_mul(
                                outb[32 * j:32 * j + G,
                                     cc * 512:(cc + 1) * 512],
                                pg[32 * j:32 * j + G, :], rss[j][:])

                    for j in range(SLAB_B):
                        nc.gpsimd.dma_start(
                            out[s * SLAB_B + j, :].rearrange(
                                "(g c) -> g c", g=G),
                            outb[32 * j:32 * j + G, :])

            with tc.tile_pool(name="wq", bufs=2) as wq_pool:
                for _rep in range(reps):
                    one_pass(out_scale=rep_scales[_rep] if rep_scales else 1.0)

    nc.compile()
    return nc


_NC_CACHE = {}


def _get_nc(b_loc=B_LOC):
    if b_loc not in _NC_CACHE:
        _NC_CACHE[b_loc] = build_nc(b_loc)
    return _NC_CACHE[b_loc]


def make_in_maps(context, query, Wq, bq, Wc, bc, Wo, bo, b_loc=B_LOC,
                 n_cores=N_CORES):
    """Host-side prep: dtype conversion, weight transposes, sharding."""
    context = np.asarray(context)
    query = np.asarray(query)
    Wq, bq = np.asarray(Wq), np.asarray(bq)
    Wc, bc = np.asarray(Wc), np.asarray(bc)
    Wo = np.asarray(Wo)
    ctx_bf = np.ascontiguousarray(context).astype(BF16)
    WqT = np.ascontiguousarray(Wq.T.astype(np.float32))
    WcT = np.ascontiguousarray(Wc.T).astype(BF16)
    WoT = np.ascontiguousarray(Wo.T).astype(BF16)
    bqc = np.ascontiguousarray(
        (bq + bc).astype(np.float32).reshape(E // 128, 128).T)
    in_maps = []
    for i in range(n_cores):
        b0 = i * b_loc
        in_maps.append(dict(
            ctx=ctx_bf[b0:b0 + b_loc].reshape(b_loc * N_CTX, CD),
            qT=np.ascontiguousarray(query[b0:b0 + b_loc].T.astype(np.float32)),
            WqT=WqT, WcT=WcT, WoT=WoT, bqc=bqc,
        ))
    return in_maps


def kernel(context, query, Wq, bq, Wc, bc, Wo, bo):
    from concourse.bass_utils import run_bass_kernel_spmd

    assert context.shape == (B_FULL, N_CTX, CD)
    nc = _get_nc()
    in_maps = make_in_maps(context, query, Wq, bq, Wc, bc, Wo, bo)
    res = run_bass_kernel_spmd(nc, in_maps, core_ids=list(range(N_CORES)))
    return np.concatenate([res.results[i]["out"] for i in range(N_CORES)],
                          axis=0)
